# revision 1
# baseline (speedup 1.0000x reference)
"""
Causal self-attention (B=4, T=2048, C=1024, H=16, D=64) on 8 trn2 NeuronCores.

Sharding: data-parallel over batch (4) x tensor-parallel over head groups (2).
Core c handles batch b = c // 2, head group g = c % 2 (8 heads, 512 features).

Mixed-precision design (cost model: fp8e4 DoubleRow matmul = 0.5 cycles/row
with 2 contraction planes packed per instruction; bf16 = 1.0; ACT exp cost is
dtype-independent):
  - Q/K projections: compensated fp8 DoubleRow: x split into x_hi + x_lo fp8
    pairs, weights fp8 (optionally + w_lo cross term, QK3): q'/k' error ~1%.
  - qt/kt stored fp8 in a paired-feature layout ([32 partitions x 2 planes]
    per head); QK^T runs as a single DoubleRow matmul per (head, key-block):
    64-dim contraction at 0.25x the f32r cost.
  - V projection: 3-term compensated fp8 DoubleRow (x_hi*wv_hi + x_hi*wv_lo +
    x_lo*wv_hi), v stored bf16 (exact to ~0.2%).
  - exp on ACT (the bottleneck engine, ~146us busy): PSUM f32 -> bf16 P with
    scale=1/8 folded into the activation. Causal wedges masked post-exp (DVE
    strided-plane triangle multiply + gpsimd memset of fully-dead blocks).
  - AV in token-major layout: out[t-sub 128, d 65] = P-block.T @ V_aug, moving
    dim = 65 features (half the cost of the feature-major form). Denominator
    via an appended ones column in V. Normalize = per-partition reciprocal +
    tensor_scalar, then DMA-transpose (bf16 XBAR) back to feature-major yt.
  - Output projection bf16 x bf16, dripped into the next t-chunk's attention
    as PE filler, like the projection chunks of the next t-chunk.
Host folds bv into the output bias (Wp @ bv), drops bk (softmax-invariant),
sums the two head-group partials per batch and adds bp.
Cost-model budget/core: ACT ~146us (bound), PE ~135us, DVE ~95us.
"""

from collections import deque

import numpy as np
import ml_dtypes

import concourse.bass as bass
import concourse.tile as tile
from concourse import bacc, mybir
from concourse.bass_utils import run_bass_kernel_spmd

B, T, C, H, D = 4, 2048, 1024, 16, 64
G = 2                 # head groups (tensor parallel)
JG = C // G           # 512 features per group
HPG = H // G          # 8 heads per group
P = 128               # partitions
TN = 512              # t-chunk width
NT = T // TN          # 4 t-chunks
MT = T // P           # 16 token 128-blocks
MJ = JG // P          # 4 feature 128-chunks per group
KP = 4                # contraction pair-chunks (1024 = 4 x [128 x 2])
NCORES = B * G
QK3 = False           # include w_lo cross term in Q/K projections

F32 = mybir.dt.float32
F8 = mybir.dt.float8e4
BF = mybir.dt.bfloat16
AF = mybir.ActivationFunctionType
DRM = mybir.MatmulPerfMode.DoubleRow
ALU = mybir.AluOpType

NP_F8 = ml_dtypes.float8_e4m3
NP_BF = ml_dtypes.bfloat16

_CACHED_NC = None


def _emit(tc, dr):
    nc = tc.nc

    with (
        tc.tile_pool(name="const", bufs=1) as cpool,
        tc.tile_pool(name="wpool", bufs=1) as wpool,
        tc.tile_pool(name="xpool", bufs=1) as xpool,
        tc.tile_pool(name="qkv", bufs=1) as qkv,
        tc.tile_pool(name="ytfp", bufs=1) as ytfp,
        tc.tile_pool(name="ptp", bufs=6) as ptp,
        tc.tile_pool(name="ytok", bufs=2) as ytokp,
        tc.tile_pool(name="recp", bufs=8) as recp,
        tc.tile_pool(name="otst", bufs=2) as otp,
        tc.tile_pool(name="stps", bufs=2, space="PSUM") as stps,
        tc.tile_pool(name="ytps", bufs=2, space="PSUM") as ytps,
        tc.tile_pool(name="fillps", bufs=2, space="PSUM") as fillps,
    ):
        tri = cpool.tile([P, P], BF, tag="tri", name="tri")
        nc.sync.dma_start(tri[:], dr["tri"])
        bqc = cpool.tile([P, MJ], F32, tag="bq", name="bqc")
        nc.gpsimd.dma_start(bqc[:], dr["bq"])

        # batched weight tiles: one [P, KP, 2, JG] tile + one DMA per tensor
        def wload(key):
            t_ = wpool.tile([P, KP, 2, JG], F8, tag=key, name=key)
            nc.scalar.dma_start(t_[:], dr[key].rearrange("k p i j -> p k i j"))
            # per-jk [P, 2, JG] views (same AP structure as standalone tiles)
            flat = t_.rearrange("p k i j -> p (k i j)")
            return [bass.AP(flat.tensor, flat.offset + jk * 2 * JG,
                            [flat.ap[0], [JG, 2], [1, JG]]) for jk in range(KP)]

        wkh, wkl = wload("wkh"), wload("wkl")
        wqh, wql = wload("wqh"), wload("wql")
        wvh, wvl = wload("wvh"), wload("wvl")
        wpt = wpool.tile([P, MJ, C], BF, tag="wp", name="wp")
        nc.scalar.dma_start(wpt[:], dr["wp"].rearrange("m p c -> p m c"))
        wp = [wpt[:, hp, :] for hp in range(MJ)]

        # x hi/lo pair tiles: one tile + two half-T DMAs per (kind, jk)
        def xload(key):
            ts = []
            for jk in range(KP):
                t_ = xpool.tile([P, 2, T], F8, tag=f"{key}{jk}", name=f"{key}{jk}")
                ts.append(t_)
            for half in range(2):
                lo, hi = half * (T // 2), (half + 1) * (T // 2)
                for jk in range(KP):
                    nc.sync.dma_start(ts[jk][:, :, lo:hi], dr[key][jk][:, :, lo:hi])
            return ts

        xh = xload("xh")
        xl = xload("xl")

        qt = [qkv.tile([P, 2, T], F8, tag=f"qt{u}", name=f"qt{u}") for u in range(2)]
        kt = [qkv.tile([P, 2, T], F8, tag=f"kt{u}", name=f"kt{u}") for u in range(2)]
        vw = HPG * (D + 1)  # 520
        v_sb = [qkv.tile([P, vw], BF, tag=f"v{s}", name=f"v{s}") for s in range(MT)]
        ytf = [ytfp.tile([P, T], BF, tag=f"ytf{hp}", name=f"ytf{hp}")
               for hp in range(MJ)]

        # ------- projection / outproj emitters, split into ~0.5us quanta -----
        def qk_chunk_parts(wh, wl, dest, bias, tn_, mj):
            u, ip = divmod(mj, 2)
            terms = []
            for jk in range(KP):
                terms.append((wh[jk], xh[jk]))
                terms.append((wh[jk], xl[jk]))
                if QK3:
                    terms.append((wl[jk], xh[jk]))
            n = len(terms)
            state = {}

            def part(t0, t1, ev):
                if t0 == 0:
                    state["ps"] = fillps.tile([P, TN], F32, tag="fill",
                                              name="fps")
                ps = state["ps"]
                for ti in range(t0, t1):
                    wt, xt = terms[ti]
                    nc.tensor.matmul(
                        ps[:], wt[:, :, mj * P:(mj + 1) * P],
                        xt[:, :, tn_ * TN:(tn_ + 1) * TN],
                        start=(ti == 0), stop=(ti == n - 1), perf_mode=DRM,
                    )
                if ev:
                    dflat = dest[u].rearrange("p i t -> p (i t)")
                    dsl = dflat[:, ip * T + tn_ * TN: ip * T + (tn_ + 1) * TN]
                    if bias is None:
                        nc.vector.tensor_scalar_mul(dsl, ps[:], 1.0 / 32.0)
                    else:
                        nc.vector.tensor_scalar(dsl, ps[:], 1.0 / 32.0, bias,
                                                ALU.mult, ALU.add)

            h_ = n // 2
            return [lambda: part(0, h_, False), lambda: part(h_, n, True)]

        def v_chunk_parts(s):
            terms = []
            for jk in range(KP):
                terms.append((xh[jk], wvh[jk]))
                terms.append((xh[jk], wvl[jk]))
                terms.append((xl[jk], wvh[jk]))
            n = len(terms)
            state = {}

            def part(t0, t1, ev):
                if t0 == 0:
                    state["ps"] = fillps.tile([P, JG], F32, tag="fill",
                                              name="fps")
                ps = state["ps"]
                for ti in range(t0, t1):
                    xt, wt = terms[ti]
                    nc.tensor.matmul(
                        ps[:], xt[:, :, s * P:(s + 1) * P], wt,
                        start=(ti == 0), stop=(ti == n - 1), perf_mode=DRM,
                    )
                if ev:
                    vv = v_sb[s].rearrange("p (h w) -> p h w", w=D + 1)
                    nc.vector.tensor_scalar_mul(
                        vv[:, :, 0:D],
                        ps.rearrange("p (h w) -> p h w", w=D), 1.0 / 32.0)
                    nc.gpsimd.memset(vv[:, :, D:D + 1], 1.0)

            return [lambda: part(0, 4, False), lambda: part(4, 8, False),
                    lambda: part(8, n, True)]

        ot_stage = {}

        def op_group_parts(tn_, cn):
            state = {}

            def part(h0, h1, ev):
                if h0 == 0:
                    state["ps"] = fillps.tile([P, TN], F32, tag="fill",
                                              name="fps")
                ps = state["ps"]
                for hp in range(h0, h1):
                    nc.tensor.matmul(
                        ps[:], wp[hp][:, cn * P:(cn + 1) * P],
                        ytf[hp][:, tn_ * TN:(tn_ + 1) * TN],
                        start=(hp == 0), stop=(hp == MJ - 1),
                    )
                if ev:
                    if cn == 0:
                        ot_stage[tn_] = otp.tile([P, C // P, TN], F32,
                                                 tag="otw", name="otw")
                    otile = ot_stage[tn_]
                    oflat = otile.rearrange("p n t -> p (n t)")
                    nc.vector.tensor_copy(oflat[:, cn * TN:(cn + 1) * TN],
                                          ps[:])
                    if cn % 2 == 1:
                        nc.sync.dma_start(
                            dr["ot"][(cn - 1) * P:(cn + 1) * P,
                                     tn_ * TN:(tn_ + 1) * TN].rearrange(
                                         "(n p) t -> p n t", p=P),
                            otile[:, cn - 1:cn + 1, :])

            return [lambda: part(0, 2, False), lambda: part(2, MJ, True)]

        filler = deque()      # projection parts for the next t-chunk (priority)
        op_filler = deque()   # deferred output-projection parts

        def drip():
            if filler:
                filler.popleft()()
            elif op_filler:
                op_filler.popleft()()

        def qk_parts_for(tn_):
            parts = []
            for mj in range(MJ):
                parts += qk_chunk_parts(wkh, wkl, kt, None, tn_, mj)
                parts += qk_chunk_parts(wqh, wql, qt, bqc[:, mj:mj + 1],
                                        tn_, mj)
            return parts

        # tn=0: K/Q for heads 0-3 plus the first two V chunks inline; the
        # rest drips into tn=0's attention.
        for mj in (0, 1):
            for p_ in qk_chunk_parts(wkh, wkl, kt, None, 0, mj):
                p_()
            for p_ in qk_chunk_parts(wqh, wql, qt, bqc[:, mj:mj + 1], 0, mj):
                p_()
        for s in (0, 1):
            for p_ in v_chunk_parts(s):
                p_()
        for s in (2, 3):
            filler.extend(v_chunk_parts(s))
        for mj in (2, 3):
            filler.extend(qk_chunk_parts(wkh, wkl, kt, None, 0, mj))
            filler.extend(qk_chunk_parts(wqh, wql, qt, bqc[:, mj:mj + 1], 0, mj))

        # ---------------- attention ----------------
        pend = {"av": None}

        for tn in range(NT):
            if tn + 1 < NT:
                filler.extend(qk_parts_for(tn + 1))
                for s in range(4 * (tn + 1), 4 * (tn + 2)):
                    filler.extend(v_chunk_parts(s))
            if tn > 0:
                for cn in range(C // P):
                    op_filler.extend(op_group_parts(tn - 1, cn))

            for h in range(HPG):
                u4, a = divmod(h, 4)
                prow = 32 * a
                npairs = 2 * tn + 2
                ytile = ytps.tile([P, 4, P], F32, tag="ytps", name="ytile")
                yflat = ytile.rearrange("p u d -> p (u d)")
                av_state = {"first": True}

                def emit_av(j, ptb_j, h_, tn_, ytile_, st, npairs_):
                    us = (2, 3) if j == 2 * tn_ + 1 else (0, 1, 2, 3)
                    for uu in us:
                        for i in range(2):
                            last = (j == npairs_ - 1 and uu == 3 and i == 1)
                            nc.tensor.matmul(
                                ytile_[:, uu, 0:D + 1],
                                ptb_j[:, i, uu * P:(uu + 1) * P],
                                v_sb[2 * j + i][:, (D + 1) * h_:
                                                (D + 1) * (h_ + 1)],
                                start=st["first"], stop=last,
                            )
                            st["first"] = False

                def emit_norm(h_, tn_, yflat_):
                    hp2, hh = divmod(h_, 2)
                    if hh == 0:
                        cur_ytoks[hp2] = [
                            ytokp.tile([P, P], BF, tag=f"yk{uu}",
                                       name=f"yk{uu}") for uu in range(4)]
                    ytoks = cur_ytoks[hp2]
                    for uu in range(4):
                        rc = recp.tile([P, 1], F32, tag="rec", name="rc")
                        with nc.allow_low_precision("softmax denom recip"):
                            nc.vector.reciprocal(
                                rc[:], yflat_[:, uu * P + D:uu * P + D + 1])
                        nc.vector.tensor_scalar_mul(
                            ytoks[uu][:, hh * D:(hh + 1) * D],
                            yflat_[:, uu * P:uu * P + D], rc[:, 0:1])
                    if hh == 1:
                        for uu in range(4):
                            nc.sync.dma_start(
                                ytf[hp2][:, tn_ * TN + uu * P:
                                         tn_ * TN + (uu + 1) * P],
                                ytoks[uu][:], transpose=True)

                for j in range(npairs):
                    c0 = 256 if j == 2 * tn + 1 else 0
                    stb = stps.tile([P, 2, TN], F32, tag="st", name="stb")
                    for i in range(2):
                        s = 2 * j + i
                        nc.tensor.matmul(
                            stb[:, i, c0:TN],
                            kt[u4][prow:prow + 32, :, s * P:(s + 1) * P],
                            qt[u4][prow:prow + 32, :, tn * TN + c0:(tn + 1) * TN],
                            start=True, stop=True, perf_mode=DRM,
                            tile_position=(prow, 0),
                        )
                    ptb = ptp.tile([P, 2, TN], BF, tag="pt", name="ptb")
                    nc.scalar.activation(ptb[:, :, c0:TN], stb[:, :, c0:TN],
                                         AF.Exp, scale=0.125)
                    if j >= 2 * tn:
                        w0 = 0 if j == 2 * tn else 256
                        tgt = bass.AP(ptb.tensor, ptb.offset + w0,
                                      [ptb.ap[0], [TN + P, 2], [1, P]])
                        tri2 = bass.AP(tri.tensor, tri.offset,
                                       [tri.ap[0], [0, 2], [1, P]])
                        nc.vector.tensor_mul(tgt, tgt, tri2)
                        nc.gpsimd.memset(ptb[:, 1:2, w0:w0 + P], 0.0)
                    # delayed AV: emit the previous pair's AV after this QK
                    drip()
                    if tn == 0:
                        drip()
                    if pend["av"] is not None:
                        pend["av"]()
                    last = (j == npairs - 1)
                    pend["av"] = (
                        lambda j_=j, p_=ptb, h_=h, tn_=tn, yt_=ytile,
                        yf_=yflat, st_=av_state, np_=npairs, last_=last:
                        (emit_av(j_, p_, h_, tn_, yt_, st_, np_),
                         emit_norm(h_, tn_, yf_) if last_ else None))

            # next t-chunk's projections must be complete before it starts
            while filler:
                drip()

        if pend["av"] is not None:
            pend["av"]()
            pend["av"] = None
        while op_filler:
            drip()
        for cn in range(C // P):
            for p_ in op_group_parts(NT - 1, cn):
                p_()


cur_ytoks = {}


def _build_program():
    nc = bacc.Bacc("TRN2", target_bir_lowering=False, debug=False,
                   num_devices=NCORES)
    dr = {
        "xh": nc.dram_tensor("xh", [KP, P, 2, T], F8, kind="ExternalInput").ap(),
        "xl": nc.dram_tensor("xl", [KP, P, 2, T], F8, kind="ExternalInput").ap(),
        "wqh": nc.dram_tensor("wqh", [KP, P, 2, JG], F8, kind="ExternalInput").ap(),
        "wql": nc.dram_tensor("wql", [KP, P, 2, JG], F8, kind="ExternalInput").ap(),
        "wkh": nc.dram_tensor("wkh", [KP, P, 2, JG], F8, kind="ExternalInput").ap(),
        "wkl": nc.dram_tensor("wkl", [KP, P, 2, JG], F8, kind="ExternalInput").ap(),
        "wvh": nc.dram_tensor("wvh", [KP, P, 2, JG], F8, kind="ExternalInput").ap(),
        "wvl": nc.dram_tensor("wvl", [KP, P, 2, JG], F8, kind="ExternalInput").ap(),
        "wp": nc.dram_tensor("wp", [MJ, P, C], BF, kind="ExternalInput").ap(),
        "bq": nc.dram_tensor("bq", [P, MJ], F32, kind="ExternalInput").ap(),
        "tri": nc.dram_tensor("tri", [P, P], BF, kind="ExternalInput").ap(),
        "ot": nc.dram_tensor("ot", [C, T], F32, kind="ExternalOutput").ap(),
    }
    with tile.TileContext(nc) as tc:
        _emit(tc, dr)
    nc.compile()
    return nc


def _get_nc():
    global _CACHED_NC
    if _CACHED_NC is None:
        _CACHED_NC = _build_program()
    return _CACHED_NC


def _perm512():
    perm = np.zeros(JG, dtype=np.int64)
    for mj in range(MJ):
        u, i = divmod(mj, 2)
        for a in range(4):
            for r in range(32):
                perm[128 * mj + 32 * a + r] = 256 * u + 64 * a + 32 * i + r
    return perm


def _hilo(arr):
    hi = arr.astype(NP_F8)
    lo = (arr - hi.astype(np.float32)).astype(NP_F8)
    return hi, lo


def _pairs(mat):
    # [C, N] -> [KP, P, 2, N] with plane i <-> contraction row 256*jk + 128*i + p
    n = mat.shape[1]
    return np.ascontiguousarray(
        mat.reshape(KP, 2, P, n).transpose(0, 2, 1, 3))


def make_in_maps(x, Wk, bk, Wq, bq, Wv, bv, Wp):
    x = np.asarray(x, dtype=np.float32)
    perm = _perm512()
    tri = np.triu(np.ones((P, P), dtype=np.float32)).astype(NP_BF)
    in_maps = []
    for core in range(NCORES):
        b, g = divmod(core, G)
        sl = slice(JG * g, JG * (g + 1))
        xT = np.ascontiguousarray(x[b].T)                       # [C, T]
        xh, xl = _hilo(_pairs(xT))
        wq_p = (32.0 * np.asarray(Wq)[sl, :][perm, :]).T        # [C, JG]
        wk_p = (32.0 * np.asarray(Wk)[sl, :][perm, :]).T
        wv_n = (32.0 * np.asarray(Wv)[sl, :]).T
        wqh, wql = _hilo(_pairs(wq_p))
        wkh, wkl = _hilo(_pairs(wk_p))
        wvh, wvl = _hilo(_pairs(wv_n))
        wp_t = np.ascontiguousarray(
            np.asarray(Wp)[:, sl].T.reshape(MJ, P, C)).astype(NP_BF)
        bq_p = np.asarray(bq)[sl][perm].reshape(MJ, P).T.copy()  # [P, MJ]
        in_maps.append({
            "xh": xh, "xl": xl,
            "wqh": wqh, "wql": wql, "wkh": wkh, "wkl": wkl,
            "wvh": wvh, "wvl": wvl,
            "wp": wp_t, "bq": np.ascontiguousarray(bq_p), "tri": tri,
        })
    return in_maps


def assemble_output(results, Wp, bv, bp):
    bias = (np.asarray(bp, np.float32)
            + np.asarray(Wp, np.float32) @ np.asarray(bv, np.float32))
    out = np.empty((B, T, C), dtype=np.float32)
    for b in range(B):
        acc = results[b * G + 0]["ot"] + results[b * G + 1]["ot"]
        out[b] = acc.T + bias
    return out


def kernel(x, Wk, bk, Wq, bq, Wv, bv, Wp, bp):
    nc = _get_nc()
    in_maps = make_in_maps(x, Wk, bk, Wq, bq, Wv, bv, Wp)
    res = run_bass_kernel_spmd(nc, in_maps, list(range(NCORES)))
    return assemble_output(res.results, Wp, bv, bp)



# revision 29
# speedup vs baseline: 1.2987x; 1.2987x over previous
"""
Causal self-attention (B=4, T=2048, C=1024, H=16, D=64) on 8 trn2 NeuronCores.

Sharding: data-parallel over batch (4) x tensor-parallel over head groups (2).
Core c handles batch b = c // 2, head group g = c % 2 (8 heads, 512 features).

Mixed-precision design (cost model: fp8e4 DoubleRow matmul = 0.5 cycles/row
with 2 contraction planes packed per instruction; bf16 = 1.0; ACT exp cost is
dtype-independent):
  - Q/K projections: compensated fp8 DoubleRow: x split into x_hi + x_lo fp8
    pairs, weights fp8 hi only: q'/k' error ~1%.
  - qt/kt stored fp8 in a paired-feature layout ([32 partitions x 2 planes]
    per head); QK^T runs as a single DoubleRow matmul per (head, key-block).
  - V projection: 3-term compensated fp8 DoubleRow (x_hi*wv_hi + x_hi*wv_lo +
    x_lo*wv_hi), v stored bf16 (exact to ~0.2%).
  - exp on ACT (the bottleneck engine): PSUM f32 -> bf16 P with scale=1/8
    folded into the activation. Causal wedges masked post-exp (DVE
    strided-plane triangle multiply); fully-dead AV blocks skipped entirely.
  - AV in token-major layout: out[t-sub 128, d 65] = P-block.T @ V_aug.
    Denominator via an appended ones column in V. Normalize = one batched
    reciprocal [P,4] + one broadcast tensor_mul per (head, t-chunk), into a
    merged [P,4,128] token-major tile; DMA-transpose (bf16 XBAR, on SP) back
    to feature-major ytf.
  - Output projection bf16 x bf16, dripped into the next t-chunk's attention
    as PE filler; output stores issued on gpsimd (SWDGE) to keep SP free.
Engine/queue split: x loads + ytf transposes on SP, weights + output stores
on Pool (SWDGE, no HWDGE contention), compute-adjacent copies on DVE.
Host folds bv into the output bias (Wp @ bv), drops bk (softmax-invariant),
sums the two head-group partials per batch and adds bp.
"""

from collections import deque

import numpy as np
import ml_dtypes

import concourse.bass as bass
import concourse.tile as tile
from concourse import bacc, mybir
from concourse.bass_utils import run_bass_kernel_spmd

B, T, C, H, D = 4, 2048, 1024, 16, 64
G = 2                 # head groups (tensor parallel)
JG = C // G           # 512 features per group
HPG = H // G          # 8 heads per group
P = 128               # partitions
TN = 512              # t-chunk width
NT = T // TN          # 4 t-chunks
MT = T // P           # 16 token 128-blocks
MJ = JG // P          # 4 feature 128-chunks per group
KP = 4                # contraction pair-chunks (1024 = 4 x [128 x 2])
NCORES = B * G

F32 = mybir.dt.float32
F8 = mybir.dt.float8e4
BF = mybir.dt.bfloat16
AF = mybir.ActivationFunctionType
DRM = mybir.MatmulPerfMode.DoubleRow
ALU = mybir.AluOpType

NP_F8 = ml_dtypes.float8_e4m3
NP_BF = ml_dtypes.bfloat16

_CACHED_NC = None


def _emit(tc, dr):
    nc = tc.nc

    def L(s):
        nc._dbg_label = s

    with (
        tc.tile_pool(name="const", bufs=1) as cpool,
        tc.tile_pool(name="wpool", bufs=1) as wpool,
        tc.tile_pool(name="xpool", bufs=1) as xpool,
        tc.tile_pool(name="qkv", bufs=1) as qkv,
        tc.tile_pool(name="ytfp", bufs=1) as ytfp,
        tc.tile_pool(name="ptp", bufs=8) as ptp,
        tc.tile_pool(name="ytok", bufs=2) as ytokp,
        tc.tile_pool(name="recp", bufs=8) as recp,
        tc.tile_pool(name="dvexp", bufs=2) as dxp,
        tc.tile_pool(name="otst", bufs=2) as otp,
        tc.tile_pool(name="stps", bufs=2, space="PSUM") as stps,
        tc.tile_pool(name="ytps", bufs=2, space="PSUM") as ytps,
        tc.tile_pool(name="fillps", bufs=2, space="PSUM") as fillps,
    ):
        L("load")
        tri = cpool.tile([P, P], BF, tag="tri", name="tri")
        nc.sync.dma_start(tri[:], dr["tri"])

        # PE p-state warmup: ~4us of dummy matmuls on tri while x/w DMAs land
        L("warmup")
        warm = stps.tile([P, 2, TN], F32, tag="st", name="warm")
        NWARM = 36
        for i in range(NWARM):
            nc.tensor.matmul(warm[:, 0, 0:P], tri[:], tri[:],
                             start=(i == 0), stop=(i == NWARM - 1))
        wsink = recp.tile([P, 4], F32, tag="rec", name="wsink")
        nc.vector.tensor_copy(wsink[:, 0:1], warm[:, 0, 0:1])

        # batched weight tiles on Pool (SWDGE; off the HWDGE + off SP/ACT)
        def wload(key):
            t_ = wpool.tile([P, KP, 2, JG], F8, tag=key, name=key)
            nc.gpsimd.dma_start(t_[:], dr[key].rearrange("k p i j -> p k i j"))
            flat = t_.rearrange("p k i j -> p (k i j)")
            return [bass.AP(flat.tensor, flat.offset + jk * 2 * JG,
                            [flat.ap[0], [JG, 2], [1, JG]]) for jk in range(KP)]

        bqc = cpool.tile([P, MJ], F32, tag="bq", name="bqc")
        nc.gpsimd.dma_start(bqc[:], dr["bq"])
        wkh = wload("wkh")
        wqh = wload("wqh")
        eye = cpool.tile([P, P], BF, tag="eye", name="eye")
        nc.gpsimd.dma_start(eye[:], dr["eye"])
        wvh = wload("wvh")
        wvl = wload("wvl")

        # x hi/lo packed tiles [P, 2(hi/lo), 2(plane), T]: one DMA per
        # (jk, t-range); first t-chunk prioritized, remainder after
        xx = [xpool.tile([P, 2, 2, T], F8, tag=f"xx{jk}", name=f"xx{jk}")
              for jk in range(KP)]
        for jk in range(KP):
            nc.sync.dma_start(xx[jk][:, :, :, 0:TN],
                              dr["xx"][jk][:, :, :, 0:TN])
        for jk in range(KP):
            nc.gpsimd.dma_start(xx[jk][:, :, :, TN:T],
                                dr["xx"][jk][:, :, :, TN:T])
        xh = [xx[jk][:, 0] for jk in range(KP)]
        xl = [xx[jk][:, 1] for jk in range(KP)]

        wpt = wpool.tile([P, MJ, C], BF, tag="wp", name="wp")
        nc.gpsimd.dma_start(wpt[:], dr["wp"].rearrange("m p c -> p m c"))
        wp = [wpt[:, hp, :] for hp in range(MJ)]

        qt = [qkv.tile([P, 2, T], F8, tag=f"qt{u}", name=f"qt{u}") for u in range(2)]
        kt = [qkv.tile([P, 2, T], F8, tag=f"kt{u}", name=f"kt{u}") for u in range(2)]
        vw = HPG * (D + 1)  # 520
        v_sb = [qkv.tile([P, vw], BF, tag=f"v{s}", name=f"v{s}") for s in range(MT)]
        ytf = [ytfp.tile([P, T], BF, tag=f"ytf{hp}", name=f"ytf{hp}")
               for hp in range(MJ)]

        # ------- projection / outproj emitters, split into ~0.5us quanta -----
        def qk_chunk_parts(wh, dest, bias, tn_, mj):
            u, ip = divmod(mj, 2)
            lab = f"proj{'Q' if bias is not None else 'K'}:tn{tn_}:mj{mj}"
            terms = []
            for jk in range(KP):
                terms.append((wh[jk], xh[jk]))
                terms.append((wh[jk], xl[jk]))
            n = len(terms)
            state = {}

            def part(t0, t1, ev):
                L(lab)
                if t0 == 0:
                    state["ps"] = fillps.tile([P, TN], F32, tag="fill",
                                              name="fps")
                ps = state["ps"]
                for ti in range(t0, t1):
                    wt, xt = terms[ti]
                    nc.tensor.matmul(
                        ps[:], wt[:, :, mj * P:(mj + 1) * P],
                        xt[:, :, tn_ * TN:(tn_ + 1) * TN],
                        start=(ti == 0), stop=(ti == n - 1), perf_mode=DRM,
                    )
                if ev:
                    dflat = dest[u].rearrange("p i t -> p (i t)")
                    dsl = dflat[:, ip * T + tn_ * TN: ip * T + (tn_ + 1) * TN]
                    if bias is None:
                        nc.vector.tensor_scalar_mul(dsl, ps[:], 1.0 / 32.0)
                    else:
                        nc.vector.tensor_scalar(dsl, ps[:], 1.0 / 32.0, bias,
                                                ALU.mult, ALU.add)

            h_ = n // 2
            return [lambda: part(0, h_, False), lambda: part(h_, n, True)]

        def v_chunk_parts(s_):
            terms = []
            for jk in range(KP):
                terms.append((xh[jk], wvh[jk]))
            for jk in range(KP):
                terms.append((xl[jk], wvh[jk]))
            for jk in range(KP):
                terms.append((xh[jk], wvl[jk]))
            n = len(terms)
            state = {}

            def part(t0, t1, ev):
                L(f"projV:s{s_}")
                if t0 == 0:
                    state["ps"] = fillps.tile([P, JG], F32, tag="fill",
                                              name="fps")
                ps = state["ps"]
                for ti in range(t0, t1):
                    xt, wt = terms[ti]
                    nc.tensor.matmul(
                        ps[:], xt[:, :, s_ * P:(s_ + 1) * P], wt,
                        start=(ti == 0), stop=(ti == n - 1), perf_mode=DRM,
                    )
                if ev:
                    vv = v_sb[s_].rearrange("p (h w) -> p h w", w=D + 1)
                    nc.vector.tensor_scalar_mul(
                        vv[:, :, 0:D],
                        ps.rearrange("p (h w) -> p h w", w=D), 1.0 / 32.0)
                    nc.gpsimd.memset(vv[:, :, D:D + 1], 1.0)

            return [lambda: part(0, 4, False), lambda: part(4, 8, False),
                    lambda: part(8, n, True)]

        ot_stage = {}

        def op_tail_a_parts(cn):
            # last chunk, hp 0-2 partial -> otile (runs mid-chunk, keeps PE warm)
            tn_ = NT - 1
            state = {}

            def part(ev):
                L(f"opA:tn{tn_}:cn{cn}")
                if not ev:
                    state["ps"] = fillps.tile([P, TN], F32, tag="fill",
                                              name="fps")
                    for hp in (0, 1, 2):
                        nc.tensor.matmul(
                            state["ps"][:], wp[hp][:, cn * P:(cn + 1) * P],
                            ytf[hp][:, tn_ * TN:(tn_ + 1) * TN],
                            start=(hp == 0), stop=(hp == 2),
                        )
                else:
                    if cn == 0:
                        ot_stage[tn_] = otp.tile([P, C // P, TN], BF,
                                                 tag="otw", name="otw")
                    otile = ot_stage[tn_]
                    oflat = otile.rearrange("p n t -> p (n t)")
                    nc.vector.tensor_copy(oflat[:, cn * TN:(cn + 1) * TN],
                                          state["ps"][:])

            return [lambda: part(False), lambda: part(True)]

        def op_tail_b(cn):
            # last chunk, hp3 on top of the staged hp 0-2 partial. Even cn:
            # DVE adds into otile; odd cn: ACT copies the hp3 partial to a
            # scratch tile stored separately (host adds) so the two chains
            # run on different engines in parallel. psum rotates over all
            # three (now idle) pools so nothing WAR-serializes.
            tn_ = NT - 1
            L(f"opB:tn{tn_}:cn{cn}")
            k = cn % 3
            if k == 0:
                ps = fillps.tile([P, TN], F32, tag="fill", name="fps")[:]
            elif k == 1:
                ps = ytps.tile([P, 4, P], F32, tag="ytps",
                               name="ytile").rearrange("p u d -> p (u d)")
            else:
                ps = stps.tile([P, 2, TN], F32, tag="st", name="stb")[:, 0, :]
            nc.tensor.matmul(
                ps, wp[3][:, cn * P:(cn + 1) * P],
                ytf[3][:, tn_ * TN:(tn_ + 1) * TN],
                start=True, stop=True,
            )
            otile = ot_stage[tn_]
            oflat = otile.rearrange("p n t -> p (n t)")
            dsl = oflat[:, cn * TN:(cn + 1) * TN]
            nc.vector.tensor_add(dsl, dsl, ps)
            if cn % 2 == 1:
                nc.sync.dma_start(
                    dr["ot"][(cn - 1) * P:(cn + 1) * P,
                             tn_ * TN:(tn_ + 1) * TN].rearrange(
                                 "(n p) t -> p n t", p=P),
                    otile[:, cn - 1:cn + 1, :])

        def op_group_parts(tn_, cn):
            state = {}

            def part(h0, h1, ev):
                L(f"op:tn{tn_}:cn{cn}")
                if h0 == 0:
                    state["ps"] = fillps.tile([P, TN], F32, tag="fill",
                                              name="fps")
                ps = state["ps"]
                for hp in range(h0, h1):
                    nc.tensor.matmul(
                        ps[:], wp[hp][:, cn * P:(cn + 1) * P],
                        ytf[hp][:, tn_ * TN:(tn_ + 1) * TN],
                        start=(hp == 0), stop=(hp == MJ - 1),
                    )
                if ev:
                    if cn == 0:
                        ot_stage[tn_] = otp.tile([P, C // P, TN], BF,
                                                 tag="otw", name="otw")
                    otile = ot_stage[tn_]
                    oflat = otile.rearrange("p n t -> p (n t)")
                    nc.vector.tensor_copy(oflat[:, cn * TN:(cn + 1) * TN],
                                          ps[:])
                    if cn % 2 == 1:
                        nc.gpsimd.dma_start(
                            dr["ot"][(cn - 1) * P:(cn + 1) * P,
                                     tn_ * TN:(tn_ + 1) * TN].rearrange(
                                         "(n p) t -> p n t", p=P),
                            otile[:, cn - 1:cn + 1, :])

            return [lambda: part(0, 2, False), lambda: part(2, MJ, True)]

        # filler: (tag, fn) deque; 'pre' parts must land before the next
        # t-chunk's attention starts, 'mid'/'post' drip freely.
        filler = deque()
        op_filler = deque()

        def drip():
            if filler:
                filler.popleft()[1]()
            elif op_filler:
                op_filler.popleft()()

        def drain_pre():
            while any(t == "pre" for t, _ in filler):
                drip()

        # tn=0: K/Q for heads 0-3 plus the first two V chunks inline; the
        # rest drips into tn=0's attention (order matches AV consumption).
        for mj in (0, 1):
            for p_ in qk_chunk_parts(wkh, kt, None, 0, mj):
                p_()
        for mj in (0, 1):
            for p_ in qk_chunk_parts(wqh, qt, bqc[:, mj:mj + 1], 0, mj):
                p_()

        # ---------------- attention ----------------
        pend_q = deque()
        pend_trans = []
        cur_ytok = {}

        def tick_trans():
            for e in pend_trans:
                e[0] -= 1
            while pend_trans and pend_trans[0][0] <= 0:
                pend_trans.pop(0)[1]()

        for tn in range(NT):
            # each chunk's V chunks + mj 2/3 K/Q drip within its OWN window
            # (front of the queue; consumed by the early heads' drips);
            # only mj 0/1 of the NEXT chunk is prepared ahead ('pre').
            for s_ in range(4 * tn, 4 * (tn + 1)):
                filler.extend(("mid", p_) for p_ in v_chunk_parts(s_))
            for mj in (2, 3):
                filler.extend(("mid", p_)
                              for p_ in qk_chunk_parts(wkh, kt, None, tn, mj))
                filler.extend(("mid", p_)
                              for p_ in qk_chunk_parts(
                                  wqh, qt, bqc[:, mj:mj + 1], tn, mj))
            if tn + 1 < NT:
                for mj in (0, 1):
                    filler.extend(("pre", p_)
                                  for p_ in qk_chunk_parts(wkh, kt, None,
                                                           tn + 1, mj))
                    filler.extend(("pre", p_)
                                  for p_ in qk_chunk_parts(
                                      wqh, qt, bqc[:, mj:mj + 1], tn + 1, mj))

            for h in range(HPG):
                u4, a = divmod(h, 4)
                prow = 32 * a
                npairs = 2 * tn + 2
                ytile = ytps.tile([P, 4, P], F32, tag="ytps", name="ytile")
                av_state = {"first": True}

                def emit_av(j, ptb_j, h_, tn_, ytile_, st, is_last):
                    L(f"av:tn{tn_}:h{h_}:j{j}")
                    ctx = tc.high_priority(offset=128)
                    ctx.__enter__()
                    if j == 2 * tn_:
                        # (uu=0, i=1) block is fully above the diagonal
                        blocks = [(0, 0), (1, 0), (1, 1), (2, 0), (2, 1),
                                  (3, 0), (3, 1)]
                    elif j == 2 * tn_ + 1:
                        # only queries 256: live; (uu=2, i=1) fully dead
                        blocks = [(2, 0), (3, 0), (3, 1)]
                    else:
                        blocks = [(uu, i) for uu in range(4) for i in range(2)]
                    lastb = blocks[-1]
                    for uu, i in blocks:
                        last = (is_last and (uu, i) == lastb)
                        nc.tensor.matmul(
                            ytile_[:, uu, 0:D + 1],
                            ptb_j[:, i, uu * P:(uu + 1) * P],
                            v_sb[2 * j + i][:, (D + 1) * h_:
                                            (D + 1) * (h_ + 1)],
                            start=st["first"], stop=last,
                        )
                        st["first"] = False
                    ctx.__exit__(None, None, None)

                def emit_norm(h_, tn_, ytile_):
                    hp2, hh = divmod(h_, 2)
                    L(f"norm:tn{tn_}:h{h_}")
                    if hh == 0:
                        cur_ytok[hp2] = ytokp.tile([P, 4, P], BF, tag="ym",
                                                   name="ym")
                    ym = cur_ytok[hp2]
                    rc = recp.tile([P, 4], F32, tag="rec", name="rc")
                    with tc.high_priority(offset=64):
                        with nc.allow_low_precision("softmax denom recip"):
                            nc.vector.reciprocal(
                                rc[:],
                                ytile_[:, :, D:D + 1].rearrange(
                                    "p u o -> p (u o)"))
                        rcb = bass.AP(rc.tensor, rc.offset,
                                      [rc.ap[0], [1, 4], [0, D]])
                        nc.vector.tensor_mul(ym[:, :, hh * D:(hh + 1) * D],
                                             ytile_[:, :, 0:D], rcb)
                    if hh == 1:
                        def do_trans(hp2_=hp2, tn2_=tn_, ym_=ym):
                            # PE transpose + DVE copy: avoids the XBAR DMA
                            # path's serialized HWDGE hops and 900ns DMA sem
                            # props; deferred 2 iterations so the DVE norm is
                            # done before the PE transposes hit the stream
                            L(f"trans:tn{tn2_}:hp{hp2_}")
                            with tc.high_priority(offset=64):
                                tps = fillps.tile([P, TN], BF, tag="fill",
                                                  name="tps")
                                for uu in range(4):
                                    nc.tensor.transpose(
                                        tps[:, uu * P:(uu + 1) * P],
                                        ym_[:, uu, :], eye[:])
                                nc.vector.tensor_copy(
                                    ytf[hp2_][:, tn2_ * TN:(tn2_ + 1) * TN],
                                    tps[:])
                        pend_trans.append([2, do_trans])

                # DVE fast-exp pairs go LAST within the head so the DVE op
                # overlaps the tail of this head's ACT exps instead of
                # stalling the next head's stps recycle
                if tn == 3:
                    jorder = [2, 0, 3, 1, 4, 5, 6, 7]
                elif tn == 2:
                    jorder = [1, 0, 2, 3, 4, 5]
                else:
                    jorder = list(range(npairs))
                for idx, j in enumerate(jorder):
                    if tn > 0 and h == 0 and idx == 3:
                        # the previous chunk's hp3 transpose has been emitted
                        # (pend at j1, tick at j2); its outproj can drip now
                        for cn in range(C // P):
                            op_filler.extend(op_group_parts(tn - 1, cn))
                    if tn == NT - 1 and h == 6 and idx == 4:
                        # hp2's transpose has been emitted (norm fires at j1
                        # via the depth-2 pend queue, transpose at j2's
                        # tick); the tail outproj's hp 0-2 partials drip in
                        for cn in range(C // P):
                            op_filler.extend(op_tail_a_parts(cn))
                    L(f"attQK:tn{tn}:h{h}:j{j}")
                    c0 = 256 if j == 2 * tn + 1 else 0
                    stb = stps.tile([P, 2, TN], F32, tag="st", name="stb")
                    with tc.high_priority(offset=256):
                        for i in range(2):
                            s = 2 * j + i
                            nc.tensor.matmul(
                                stb[:, i, c0:TN],
                                kt[u4][prow:prow + 32, :, s * P:(s + 1) * P],
                                qt[u4][prow:prow + 32, :,
                                       tn * TN + c0:(tn + 1) * TN],
                                start=True, stop=True, perf_mode=DRM,
                                tile_position=(prow, 0),
                            )
                    ptb = ptp.tile([P, 2, TN], BF, tag="pt", name="ptb")
                    if (tn == 3 and j <= 1) or (tn == 2 and j == 0):
                        # fast-exp on DVE for some off-diagonal blocks:
                        # bf16-bits(exp(s/8)) ~ int16(s*C1 + C2); softmax's
                        # num/denom error correlation cancels the ~3%
                        # sawtooth to ~1e-3 at the output. Rebalances the
                        # bottleneck ACT engine onto DVE slack.
                        L(f"dvexp:tn{tn}:h{h}:j{j}")
                        C1 = 0.125 * 128.0 / 0.6931471805599453
                        C2 = 127.0 * 128.0 - 486411.0 / 65536.0
                        with tc.high_priority(offset=192):
                            nc.vector.tensor_scalar(
                                ptb[:].bitcast(mybir.dt.int16), stb[:],
                                C1, C2, ALU.mult, ALU.add)
                    else:
                        L(f"exp:tn{tn}:h{h}:j{j}")
                        nc.scalar.activation(ptb[:, :, c0:TN],
                                             stb[:, :, c0:TN],
                                             AF.Exp, scale=0.125)
                    if j >= 2 * tn:
                        w0 = 0 if j == 2 * tn else 256
                        tgt = bass.AP(ptb.tensor, ptb.offset + w0,
                                      [ptb.ap[0], [TN + P, 2], [1, P]])
                        tri2 = bass.AP(tri.tensor, tri.offset,
                                       [tri.ap[0], [0, 2], [1, P]])
                        L(f"mask:tn{tn}:h{h}:j{j}")
                        nc.vector.tensor_mul(tgt, tgt, tri2)
                    # delayed AV: emit the previous pair's AV after this QK.
                    # pend MUST fire before drips: op_filler parts read ytf
                    # written by the transposes that pend's emit_norm issues.
                    # AV/norm run 1-2 pairs behind exp (depth 2 while h==0
                    # so the new V chunks' emission deadlines spread over
                    # more drip slots); drips fill PE with proj/outproj work
                    drip()
                    cap = 2 if h == 0 else 1
                    while len(pend_q) > cap:
                        pend_q.popleft()()
                    tick_trans()
                    ndrip = (4 if tn == 0 and h <= 1 else
                             3 if tn > 0 and h == 1 else 2)
                    for _ in range(ndrip - 1):
                        drip()
                    last = (idx == npairs - 1)
                    pend_q.append(
                        lambda j_=j, p_=ptb, h_=h, tn_=tn, yt_=ytile,
                        st_=av_state, last_=last:
                        (emit_av(j_, p_, h_, tn_, yt_, st_, last_),
                         emit_norm(h_, tn_, yt_) if last_ else None))

            # next t-chunk's mj 0/1 projections + V must complete before it
            # starts; mj 2/3 keep dripping during its first heads.
            drain_pre()

        while pend_q:
            pend_q.popleft()()
        while pend_trans:
            pend_trans.pop(0)[1]()
        while filler or op_filler:
            drip()
        for cn in range(C // P):
            op_tail_b(cn)


def _build_program():
    nc = bacc.Bacc("TRN2", target_bir_lowering=False, debug=False,
                   num_devices=NCORES)
    dr = {
        "xx": nc.dram_tensor("xx", [KP, P, 2, 2, T], F8, kind="ExternalInput").ap(),
        "wqh": nc.dram_tensor("wqh", [KP, P, 2, JG], F8, kind="ExternalInput").ap(),
        "wkh": nc.dram_tensor("wkh", [KP, P, 2, JG], F8, kind="ExternalInput").ap(),
        "wvh": nc.dram_tensor("wvh", [KP, P, 2, JG], F8, kind="ExternalInput").ap(),
        "wvl": nc.dram_tensor("wvl", [KP, P, 2, JG], F8, kind="ExternalInput").ap(),
        "wp": nc.dram_tensor("wp", [MJ, P, C], BF, kind="ExternalInput").ap(),
        "bq": nc.dram_tensor("bq", [P, MJ], F32, kind="ExternalInput").ap(),
        "tri": nc.dram_tensor("tri", [P, P], BF, kind="ExternalInput").ap(),
        "eye": nc.dram_tensor("eye", [P, P], BF, kind="ExternalInput").ap(),
        "ot": nc.dram_tensor("ot", [C, T], BF, kind="ExternalOutput").ap(),
    }
    with tile.TileContext(nc) as tc:
        _emit(tc, dr)
    nc.compile()
    return nc


def _get_nc():
    global _CACHED_NC
    if _CACHED_NC is None:
        _CACHED_NC = _build_program()
    return _CACHED_NC


def _perm512():
    perm = np.zeros(JG, dtype=np.int64)
    for mj in range(MJ):
        u, i = divmod(mj, 2)
        for a in range(4):
            for r in range(32):
                perm[128 * mj + 32 * a + r] = 256 * u + 64 * a + 32 * i + r
    return perm


def _hilo(arr):
    hi = arr.astype(NP_F8)
    lo = (arr - hi.astype(np.float32)).astype(NP_F8)
    return hi, lo


def _pairs(mat):
    # [C, N] -> [KP, P, 2, N] with plane i <-> contraction row 256*jk + 128*i + p
    n = mat.shape[1]
    return np.ascontiguousarray(
        mat.reshape(KP, 2, P, n).transpose(0, 2, 1, 3))


def make_in_maps(x, Wk, bk, Wq, bq, Wv, bv, Wp):
    x = np.asarray(x, dtype=np.float32)
    perm = _perm512()
    tri = np.triu(np.ones((P, P), dtype=np.float32)).astype(NP_BF)
    eye = np.eye(P, dtype=np.float32).astype(NP_BF)
    in_maps = []
    for core in range(NCORES):
        b, g = divmod(core, G)
        sl = slice(JG * g, JG * (g + 1))
        xT = np.ascontiguousarray(x[b].T)                       # [C, T]
        xh, xl = _hilo(_pairs(xT))
        xx = np.ascontiguousarray(np.stack([xh, xl], axis=2))   # [KP,P,2,2,T]
        wq_p = (32.0 * np.asarray(Wq)[sl, :][perm, :]).T        # [C, JG]
        wk_p = (32.0 * np.asarray(Wk)[sl, :][perm, :]).T
        wv_n = (32.0 * np.asarray(Wv)[sl, :]).T
        wqh, _ = _hilo(_pairs(wq_p))
        wkh, _ = _hilo(_pairs(wk_p))
        wvh, wvl = _hilo(_pairs(wv_n))
        wp_t = np.ascontiguousarray(
            np.asarray(Wp)[:, sl].T.reshape(MJ, P, C)).astype(NP_BF)
        bq_p = np.asarray(bq)[sl][perm].reshape(MJ, P).T.copy()  # [P, MJ]
        in_maps.append({
            "xx": xx,
            "wqh": wqh, "wkh": wkh,
            "wvh": wvh, "wvl": wvl,
            "wp": wp_t, "bq": np.ascontiguousarray(bq_p), "tri": tri,
            "eye": eye,
        })
    return in_maps


def assemble_output(results, Wp, bv, bp):
    bias = (np.asarray(bp, np.float32)
            + np.asarray(Wp, np.float32) @ np.asarray(bv, np.float32))
    out = np.empty((B, T, C), dtype=np.float32)
    for b in range(B):
        acc = (results[b * G + 0]["ot"].astype(np.float32)
               + results[b * G + 1]["ot"].astype(np.float32))
        out[b] = acc.T + bias
    return out


def kernel(x, Wk, bk, Wq, bq, Wv, bv, Wp, bp):
    nc = _get_nc()
    in_maps = make_in_maps(x, Wk, bk, Wq, bq, Wv, bv, Wp)
    res = run_bass_kernel_spmd(nc, in_maps, list(range(NCORES)))
    return assemble_output(res.results, Wp, bv, bp)
